# revision 67
# baseline (speedup 1.0000x reference)
"""Trainium2 Bass kernel for nn_DomainGCN (4-layer GCN + MLP head), 8 NeuronCores.

Strategy (graph/data parallel, per sharding hint):
  - Nodes sharded contiguously: core r owns rows [r*6272, (r+1)*6272) (padded).
  - Reformulation: Agg(h@W) with per-edge weight dis[src]*dis[dst] and
    self-loops as a dedicated per-block self tile (weight dis[j]^2):
      z = h @ W            (dense, f32r matmuls, feature-major h in SBUF)
      z -> zdt -> AllGather (every core gets the full z, node-major, in DRAM)
      agg = S.T @ z[src]   (dma_gather of edge messages + one-hot-weighted
                            matmuls accumulating per 128-dst-node block in PSUM)
      h' = relu(agg + b)   (DVE add + ACT relu), PE-transposed back to
                            feature-major for the next dense matmul.
  - v3 (this session, 2811965ns -> ~1882535ns on HW):
    * SOFTWARE-PIPELINED emission: agg(l-1, b) and dense(l, b) interleaved
      per block; AllGather pieces of layer l fire mid-aggregation.
    * z payload fp8e4m3 for layers 1-3 (layer 3 stored scaled x16, layer 2
      x8 -- scales fold into W/b host-side via relu positive homogeneity,
      zero runtime cost); bf16 for layer 4 (256B dma_gather row minimum).
    * aggregation matmuls use DoubleRow perf mode on the fp8 layers (two
      128-edge tiles per instruction).
    * S one-hot tiles are built ON-CHIP (DVE: mask = iota==DC, S = mask*WW,
      batched per block with stride-0 broadcast APs) from [128, TT] f16
      metadata instead of streaming ~19MB/layer of host-built one-hots.
    * gather num_idxs statically trimmed to the worst-core true count
      (gbuf slots memset once so never-written tail rows stay finite).
    * deep prefetch: 6 gather buffers, K=6 groups preloaded at each phase
      boundary to ride out the in-flight tail AllGather piece.
    * asymmetric AG pieces [31, 18] blocks: the piece-1 segments then fit
      one 7-tile SWDGE call per block (3 gather calls/block, not 4) --
      GpSimd descriptor generation (994ns fixed/call) is the top bottleneck.
  - dma_gather indices are int16; AllGather runs as two piece collectives
    so gathered row ids stay < 32768.
  - Layer-4 aggregation feeds the MLP head block-by-block (short tail).
"""

import os
import math
import numpy as np

import concourse.bass as bass
import concourse.bacc as bacc
import concourse.mybir as mybir
import concourse.tile as tile
from concourse import bass_utils

# problem constants (hardcoded per task contract)
N, E = 50000, 800000
DIN = DH = 512
DE, MH, NCLS = 10, 64, 20
NCORES = 8
NB = 49                   # dst blocks of 128 nodes per core
SHARD = NB * 128          # 6272
NPAD = SHARD * NCORES     # 50176
KC = 4                    # 128-feature chunks of 512

f32 = mybir.dt.float32
f32r = mybir.dt.float32r
f16 = mybir.dt.float16
bf16 = mybir.dt.bfloat16
fp8 = mybir.dt.float8e4
i16 = mybir.dt.int16

# z-path dtype per layer (gather payload + AllGather + self tiles)
# Layers 1-3 fp8: z_l is stored scaled by ZSCALE[l] to center values in
# e4m3's range (z3 mean |z| ~0.05 underflows unscaled).  The scales fold
# into the dense weights/biases at zero runtime cost: relu is positively
# homogeneous, so h_l' = c_l*h_l propagates through W_{l+1}' =
# (c_{l+1}/c_l) * W_{l+1}.
ZDTL = {1: fp8, 2: fp8, 3: fp8, 4: bf16}
ZSCALE = {1: 1.0, 2: 8.0, 3: 16.0, 4: 1.0}
ZNPL = {l: mybir.dt.np(dt) for l, dt in ZDTL.items()}
DOUT = {1: DH, 2: DH, 3: DH, 4: 128}

# AllGather pieces (block-aligned): fired after dense tiles 30 / 48 of each
# layer.  (A 3-piece split was tried and regressed: collective time is
# floor-dominated at these sizes, so extra pieces cost more than the smaller
# exposed tail saves.)
# [31,18] asymmetric split: piece-1 segments then fit ONE 7-tile gather call
# per block (3 SWDGE calls/block instead of 4) -- the 994ns-fixed cost
# descriptor generation on GpSimd is the top bottleneck.  (This split
# regressed +53us in the old S-load/K=3 structure from DMA contention with
# the late piece-0 collective; DMA has since eased ~90%->~70% busy.)
PBLK = [31, 18]
PROW = [b * 128 for b in PBLK]            # rows per piece
PSTART = [0, PROW[0]]
NPIECE = 2

# Each piece ships as one sub-collective.  (Splitting piece-1 into two subs
# [31,9,9] -- smaller tail collective, same gather segments -- was tried and
# measured +25us: the per-collective floor outweighs the earlier landing.)
SUBBLK = [31, 18]
SUBPIECE = [0, 1]                          # which zf piece each sub writes
SUBROW = [b * 128 for b in SUBBLK]
SUBSTART = [0, 3968]                       # z_loc row range starts
SUBBASE = [0, 0]                           # dest row base within the piece zf
NSUB = 2

# dst blocks are processed one per group.  (Groups of 2 were tried to halve
# the 994ns-fixed-cost SWDGE gather calls and measured +89us WORSE: a
# block's compute then waits on the whole group's gathers -- per-block gbuf
# sync granularity is load-bearing for the gather/compute overlap.)
GROUPS = [(b,) for b in range(NB)]
NG = len(GROUPS)

LAST_RESULT = None        # BassKernelResults of the most recent run (for test.py)
_BUILD_CACHE = {}


# ---------------------------------------------------------------- host prep

def _host_prep(x, edge_index):
    src = edge_index[0].astype(np.int64)
    dst = edge_index[1].astype(np.int64)
    deg = np.bincount(dst, minlength=N).astype(np.float32) + 1.0
    dis = (1.0 / np.sqrt(deg)).astype(np.float32)

    sa, da = src, dst
    w = (dis[sa] * dis[da]).astype(np.float32)

    # Segments: src piece membership.  AllGather runs as NPIECE block-aligned
    # piece collectives, so the gathered tensors are zf[j] with rank-major
    # pieces; row ids stay < 32768 (int16-safe) with no extra split.
    r = da // SHARD
    b = (da % SHARD) // 128
    so = sa % SHARD
    k = np.searchsorted(np.array(SUBSTART[1:]), so, side="right")  # sub idx
    s = np.array(SUBPIECE)[k]                                      # piece idx
    row = (np.array(SUBBASE)[k] + (sa // SHARD) * np.array(SUBROW)[k]
           + (so - np.array(SUBSTART)[k]))         # row in zf[s]
    order = np.lexsort((row, s, b, r))
    row_s, da_s, w_s = row[order], da[order], w[order]
    key = (r * NB + b) * NPIECE + s
    ks = key[order]
    counts = np.bincount(ks, minlength=NCORES * NB * NPIECE)\
        .reshape(NCORES, NB, NPIECE)
    starts = np.zeros(NCORES * NB * NPIECE + 1, np.int64)
    np.cumsum(counts.reshape(-1), out=starts[1:])

    # uniform program structure: per (block, seg) tile count = max over cores
    T_seg = np.ceil(counts / 128.0).astype(np.int64).max(axis=0)  # [NB, NPIECE]
    TT = int(T_seg.sum()) + NB   # +1 self tile per block

    gidx, DC, WW = [], [], []
    for rr in range(NCORES):
        idx16 = np.zeros(TT * 128, np.int16)
        dcol = np.full(TT * 128, -1, np.int64)
        wcol = np.zeros(TT * 128, np.float32)
        cur = 0
        for B in GROUPS:
            # self tiles first (tile k of the group = block B[k]'s own
            # diag(dis^2); msg row p comes from local z row bb*128+p),
            # then per segment the blocks' edge tiles back to back.
            for bb in B:
                gnode = rr * SHARD + bb * 128 + np.arange(128)
                ok = gnode < N
                dcol[cur:cur + 128][ok] = np.arange(128)[ok]
                wcol[cur:cur + 128][ok] = (dis[gnode[ok]] ** 2)
                cur += 128
            for ss in range(NPIECE):
                for bb in B:
                    k = (rr * NB + bb) * NPIECE + ss
                    n = counts[rr, bb, ss]
                    lo, hi = starts[k], starts[k] + n
                    idx16[cur:cur + n] = row_s[lo:hi].astype(np.int16)
                    dcol[cur:cur + n] = da_s[lo:hi] - rr * SHARD - bb * 128
                    wcol[cur:cur + n] = w_s[lo:hi]
                    cur += int(T_seg[bb, ss]) * 128  # pads: idx 0, dst -1, w 0
        gidx.append(np.tile(idx16.reshape(-1, 16).T, (8, 1)))       # [128, TT*8]
        # S tiles are built ON-CHIP (DVE: iota==DC mask, then *WW) from the
        # per-edge (dst column, weight) metadata -- [128, TT] f16 each, edge
        # p of tile e at [p, e].  Pads have DC=-1 (mask never matches).
        DC.append(np.ascontiguousarray(
            dcol.reshape(TT, 128).T).astype(np.float16))
        WW.append(np.ascontiguousarray(
            wcol.reshape(TT, 128).T).astype(np.float16))

    return {
        "T_seg": T_seg, "TT": TT, "gidx": gidx, "DC": DC, "WW": WW,
        "cmax": counts.max(axis=0),   # true idx count per (block, seg)
    }


def _chunk_w(W):
    """[K, M] -> [128, (K//128)*M] with k-chunk c at cols [c*M, (c+1)*M)."""
    K, M = W.shape
    return np.ascontiguousarray(
        W.reshape(K // 128, 128, M).transpose(1, 0, 2).reshape(128, -1)
    ).astype(np.float16)


# ---------------------------------------------------------------- kernel build

def _build(T_seg_t, TT, cmax_t):
    T_seg = np.asarray(T_seg_t).reshape(NB, NPIECE)
    cmax = np.asarray(cmax_t).reshape(NB, NPIECE)
    # group tile counts: len(B) self tiles + per-seg per-block edge tiles
    GT = [len(B) + int(T_seg[list(B)].sum()) for B in GROUPS]
    GTMAX = max(GT)

    nc = bacc.Bacc("TRN2", target_bir_lowering=False, debug=False,
                   num_devices=NCORES, num_swdge_queues=4)

    dt_in = {}

    def din(name, shape, dt):
        dt_in[name] = nc.dram_tensor(name, shape, dt, kind="ExternalInput")
        return dt_in[name]

    xT = din("xT", [DIN, SHARD], f16)
    Wd = {l: din(f"W{l}", [128, KC * DOUT[l]], f16) for l in (1, 2, 3, 4)}
    Bd = {l: din(f"B{l}", [128, DOUT[l]], f32) for l in (1, 2, 3, 4)}
    Brd = {l: din(f"Br{l}", [1, DOUT[l]], f16) for l in (1, 2, 3, 4)}
    M1p = din("M1p", [128, MH], f16)
    M2d = din("M2d", [MH, MH], f16)
    M3d = din("M3d", [MH, NCLS], f16)
    MB1 = din("MB1", [MH, 1], f32)
    MB2 = din("MB2", [MH, 1], f32)
    MB3b = din("MB3b", [128, NCLS], f32)
    ident_c = din("ident_c", [128, 128], f32)
    gidx = din("gidx", [128, TT * 8], i16)
    DCd = din("DCd", [128, TT], f16)
    WWd = din("WWd", [128, TT], f16)
    iota_c = din("iota_c", [128, 128], f16)
    out = nc.dram_tensor("out", [SHARD, NCLS], f32, kind="ExternalOutput")

    # persistent SBUF (h and dense weights in fp16: 10-bit mantissa keeps the
    # dense path's error contribution ~0.05%/elem — bf16 was measured too
    # lossy — while halving the dominant SBUF footprint vs f32r)
    hT = [nc.alloc_sbuf_tensor(f"hT{k}", [128, SHARD], f16).ap() for k in range(KC)]
    W_sb = {p: nc.alloc_sbuf_tensor(f"W_sb{p}", [128, KC * DH], f16).ap()
            for p in (0, 1)}
    W4_sb = nc.alloc_sbuf_tensor("W4_sb", [128, KC * 128], f16).ap()
    ident_sb = nc.alloc_sbuf_tensor("ident_sb", [128, 128], f32).ap()
    DC_sb = nc.alloc_sbuf_tensor("DC_sb", [128, TT], f16).ap()
    WW_sb = nc.alloc_sbuf_tensor("WW_sb", [128, TT], f16).ap()
    iota_sb = nc.alloc_sbuf_tensor("iota_sb", [128, 128], f16).ap()
    Bb_sb = {l: nc.alloc_sbuf_tensor(f"Bb{l}", [128, DOUT[l]], f32).ap()
             for l in (1, 2, 3, 4)}
    Brow_sb = {l: nc.alloc_sbuf_tensor(f"Brow{l}", [1, DOUT[l]], f16).ap()
               for l in (1, 2, 3, 4)}
    ones_sb = nc.alloc_sbuf_tensor("ones_sb", [1, 128], f16).ap()
    M1_sb = nc.alloc_sbuf_tensor("M1_sb", [128, MH], f16).ap()
    M2_sb = nc.alloc_sbuf_tensor("M2_sb", [MH, MH], f16).ap()
    M3_sb = nc.alloc_sbuf_tensor("M3_sb", [MH, NCLS], f16).ap()
    MB1_sb = nc.alloc_sbuf_tensor("MB1_sb", [MH, 1], f32).ap()
    MB2_sb = nc.alloc_sbuf_tensor("MB2_sb", [MH, 1], f32).ap()
    MB3_sb = nc.alloc_sbuf_tensor("MB3_sb", [128, NCLS], f32).ap()

    # DRAM z buffers, one set per layer (race-free pipelining across layers)
    z_loc = {l: nc.dram_tensor(f"zloc{l}", [SHARD, DOUT[l]], ZDTL[l],
                               kind="Internal").ap() for l in (1, 2, 3, 4)}


    zf = {l: tuple(nc.dram_tensor(f"zf{l}_{j}", [NCORES * PROW[j], DOUT[l]],
                                  ZDTL[l], kind="Internal",
                                  addr_space="Shared").ap()
                   for j in range(NPIECE))
          for l in (1, 2, 3, 4)}

    rg = [list(range(NCORES))]

    # column offsets of each group in gidx/DC/WW, and within-group tile
    # offsets: [selfs][seg0: blocks][seg1: blocks]
    gcol_of = np.concatenate([[0], np.cumsum(GT)]).astype(int)

    def seg_off(g, j):
        """Offset of seg j's tile run within group g (all blocks)."""
        B = GROUPS[g]
        return len(B) + int(T_seg[list(B), :j].sum())

    def blk_off(g, j, k):
        """Offset of block B[k]'s seg-j tile run within group g."""
        B = GROUPS[g]
        return seg_off(g, j) + int(T_seg[list(B[:k]), j].sum())

    with tile.TileContext(nc) as tc:
        with (
            tc.tile_pool(name="meta", bufs=8) as meta,
            tc.tile_pool(name="gp", bufs=6) as gp,
            tc.tile_pool(name="sp", bufs=6) as sp,
            tc.tile_pool(name="mk", bufs=2) as mk,
            tc.tile_pool(name="zp", bufs=3) as zp,
            tc.tile_pool(name="hp", bufs=3) as hp,
            tc.tile_pool(name="mp", bufs=3) as mp,
            tc.tile_pool(name="ps", bufs=2, space="PSUM") as ps,
            tc.tile_pool(name="psa", bufs=3, space="PSUM") as psa,
            tc.tile_pool(name="pst", bufs=2, space="PSUM") as pst,
        ):
            # ---- constant / weight loads (hT in column chunks so dense L1
            # can start early)
            # startup loads split across both HWDGE rings (scalar is otherwise
            # idle until the first agg phase) so dense L1 and the first
            # AllGather fire sooner
            nc.scalar.dma_start(W_sb[1], Wd[1].ap())
            nc.sync.dma_start(ident_sb, ident_c.ap())
            nc.scalar.dma_start(DC_sb, DCd.ap())
            nc.scalar.dma_start(WW_sb, WWd.ap())
            nc.scalar.dma_start(iota_sb, iota_c.ap())
            CCH = SHARD // 4
            for c in range(4):
                for k in range(KC):
                    eng = nc.sync if k % 2 == 0 else nc.scalar
                    eng.dma_start(
                        hT[k][:, c * CCH:(c + 1) * CCH],
                        xT[k * 128:(k + 1) * 128, c * CCH:(c + 1) * CCH])
            nc.sync.dma_start(W_sb[0], Wd[2].ap())
            nc.sync.dma_start(W4_sb, Wd[4].ap())
            for l in (1, 2, 3, 4):
                nc.sync.dma_start(Bb_sb[l], Bd[l].ap())
                nc.sync.dma_start(Brow_sb[l], Brd[l].ap())
            nc.vector.memset(ones_sb, 1.0)
            nc.sync.dma_start(M1_sb, M1p.ap())
            nc.sync.dma_start(M2_sb, M2d.ap())
            nc.sync.dma_start(M3_sb, M3d.ap())
            nc.sync.dma_start(MB1_sb, MB1.ap())
            nc.sync.dma_start(MB2_sb, MB2.ap())
            nc.sync.dma_start(MB3_sb, MB3b.ap())

            # SWDGE queue load balancing (greedy by tile count)
            qload = [0, 0, 0, 0]

            # zero the gather-buffer slots once: trimmed tail rows are never
            # gathered, and uninitialized SBUF could hold fp8 NaN patterns
            for _ in range(6):
                zt = gp.tile([128, GTMAX, DH], fp8, tag="gbuf", name="gbuf8")
                nc.vector.memset(zt[:], 0)

            def gbuf_tile(l):
                # All layers share one pool slot family (same bytes per slot):
                #   l=1..3: fp8  [128, GTMAX, 512] -> group tiles at 512B
                #   l=4:    bf16 [128, 2*GTMAX, 128] -> group tiles at 256B
                if l <= 3:
                    return gp.tile([128, GTMAX, DH], fp8, tag="gbuf",
                                   name="gbuf8")
                return gp.tile([128, 2 * GTMAX, 128], bf16, tag="gbuf",
                               name="gbuf4")

            def dense_tile(l, t):
                """z_l[t] = hT[:, t] @ W_l -> zdt -> DRAM z_loc."""
                Dout = DOUT[l]
                wsb = W_sb[l % 2] if l < 4 else W4_sb
                zps = ps.tile([128, DH], f32, tag="zps")
                for k in range(KC):
                    nc.tensor.matmul(
                        zps[:, 0:Dout],
                        hT[k][:, t * 128:(t + 1) * 128],
                        wsb[:, k * Dout:(k + 1) * Dout],
                        start=(k == 0), stop=(k == KC - 1),
                    )
                zsb = zp.tile([128, Dout], ZDTL[l], tag=f"zsb_{l}")
                nc.scalar.activation(zsb[:], zps[:, 0:Dout],
                                     mybir.ActivationFunctionType.Copy)
                nc.sync.dma_start(z_loc[l][t * 128:(t + 1) * 128, :], zsb[:])

            def fire_ag(l, k):
                """Fire sub-collective k (writes a row slice of its piece)."""
                j = SUBPIECE[k]
                nc.gpsimd.collective_compute(
                    "AllGather", mybir.AluOpType.bypass,
                    replica_groups=rg,
                    ins=[z_loc[l][SUBSTART[k]:SUBSTART[k] + SUBROW[k], :]],
                    outs=[zf[l][j][SUBBASE[k]:SUBBASE[k]
                                   + NCORES * SUBROW[k], :]],
                )

            def agg_load(l, g):
                """gbuf alloc + idx/S/self loads + seg-0 gathers for group g."""
                Dout = DOUT[l]
                B = GROUPS[g]
                TG = GT[g]
                col = int(gcol_of[g])
                gbuf = gbuf_tile(l)
                idx_sb = meta.tile([128, TG * 8], i16, tag="idx")
                nc.sync.dma_start(idx_sb[:], gidx.ap()[:, col * 8:(col + TG) * 8])
                # Build S on-chip (saves ~19MB of HBM one-hot traffic per
                # layer): mask = (iota == DC), S = mask * WW, batched over
                # the whole group's TG tiles with stride-0 broadcast APs.
                # (Caching built S in per-block DRAM tensors for layers 2-3
                # was tried twice and regressed both times (+103us, +35us):
                # DVE runs in parallel slack here -- GatherGen+DMA bind, and
                # the cache traffic re-pressures DMA.)
                sdt = fp8 if l <= 3 else bf16
                S_sb = sp.tile([128, TG, 128], sdt, tag="S8" if l <= 3 else "S")
                msk = mk.tile([128, TG, 128], f16, tag="msk")
                i3 = iota_sb.rearrange("p (a d) -> p a d", a=1)      # [128,1,128]
                d3 = DC_sb[:, col:col + TG]\
                    .rearrange("p (t a) -> p t a", a=1)              # [128,TG,1]
                ib, db = bass.broadcast_tensor_aps(i3, d3)
                nc.vector.tensor_tensor(msk[:], ib, db,
                                        mybir.AluOpType.is_equal)
                w3 = WW_sb[:, col:col + TG]\
                    .rearrange("p (t a) -> p t a", a=1)              # [128,TG,1]
                mb_, wb = bass.broadcast_tensor_aps(msk[:], w3)
                nc.vector.tensor_tensor(S_sb[:], mb_, wb,
                                        mybir.AluOpType.mult)

                # self tiles: one contiguous copy of the group's own z rows
                nb = len(B)
                nc.sync.dma_start(gbuf[:, 0:nb, :],
                                  z_loc[l][B[0] * 128:(B[0] + nb) * 128, :]
                                  .rearrange("(a p) d -> p a d", a=nb))
                ctx = (l, g, Dout, gbuf, idx_sb, S_sb)
                agg_gather(ctx, 0)
                return ctx

            def agg_gather(ctx, j):
                """Emit the seg-j gathers for group g (all blocks)."""
                l, g, Dout, gbuf, idx_sb, S_sb = ctx
                off = seg_off(g, j)
                # SWDGE descriptor ring holds ~1024 descs/queue (ucode
                # constant — it does NOT grow with dynamic_dma_scratch_size;
                # bigger calls hang the device).  Cap at 7 tiles (896 idxs).
                # num_idxs is trimmed to the worst-core true count (the tail
                # pads are never gathered; their S rows are zero and gbuf
                # slots are memset once at startup, so stale rows are finite).
                left = int(T_seg[list(GROUPS[g]), j].sum())
                rows = int(sum(cmax[b, j] for b in GROUPS[g]))
                while left > 0:
                    sub = min(7, left)
                    n = min(sub * 128, rows)
                    nt = (n + 127) // 128
                    qn = qload.index(min(qload))
                    qload[qn] += sub
                    nc.gpsimd.dma_gather(
                        gbuf[:, off:off + nt, :],
                        zf[l][j],
                        idx_sb[:, off * 8:(off + nt) * 8],
                        num_idxs=n, num_idxs_reg=n, elem_size=Dout,
                        queue_num=qn,
                    )
                    off += sub
                    left -= sub
                    rows -= n

            def agg_compute(ctx, k):
                """S.T @ gbuf accumulation + epilogue for block B[k] of g."""
                l, g, Dout, gbuf, idx_sb, S_sb = ctx
                b = GROUPS[g][k]
                # tile runs of this block: [self] + per-seg slices; merge
                # adjacent contiguous runs so DoubleRow pairs span them
                runs = [(k, 1)] + [
                    (blk_off(g, j, k), int(T_seg[b, j]))
                    for j in range(NPIECE)
                ]
                merged = [runs[0]]
                for off, cnt in runs[1:]:
                    po, pc = merged[-1]
                    if off == po + pc:
                        merged[-1] = (po, pc + cnt)
                    else:
                        merged.append((off, cnt))
                runs = merged
                # layer 4 only has 10 real output features (z4 rows padded to
                # 128 for the 256B gather floor): stream just 16 matmul cols
                Dm = 16 if l == 4 else Dout
                aps = psa.tile([128, DH], f32, tag="aps")
                first = True
                for off, cnt in runs:
                    e = off
                    while e < off + cnt:
                        if l <= 3 and e + 1 < off + cnt:
                            # fp8: DoubleRow packs two 128-edge tiles per
                            # instruction (contracts 256 edges)
                            nc.tensor.matmul(
                                aps[:, 0:Dm],
                                S_sb[:, e:e + 2, :], gbuf[:, e:e + 2, :],
                                start=first, stop=False,
                                perf_mode=mybir.MatmulPerfMode.DoubleRow,
                            )
                            e += 2
                        else:
                            nc.tensor.matmul(
                                aps[:, 0:Dm], S_sb[:, e, :],
                                gbuf[:, e, 0:Dm],
                                start=first, stop=False,
                            )
                            e += 1
                        first = False
                # bias folded into the accumulation as a rank-1 matmul
                # (ones.T @ bias_row) so the relu can read PSUM directly
                nc.tensor.matmul(
                    aps[:, 0:Dm], ones_sb[0:1, 0:128],
                    Brow_sb[l][0:1, 0:Dm],
                    start=False, stop=True,
                )

                # epilogue: h = relu(agg + b) straight from PSUM (bias is
                # already accumulated); transpose back to feature-major
                hsb = hp.tile([128, Dm], f32, tag="hsb")
                nc.scalar.activation(hsb[:], aps[:, 0:Dm],
                                     mybir.ActivationFunctionType.Relu)
                if l == 4:
                    # only rows 0:16 of hT[0] are live; M1p rows 10:128 are
                    # zero so the stale layer-3 rows below don't contribute
                    tps = pst.tile([128, 128], f32, tag="tps")
                    nc.tensor.transpose(tps[0:Dm, :], hsb[:, 0:Dm], ident_sb)
                    nc.scalar.activation(hT[0][0:Dm, b * 128:(b + 1) * 128],
                                         tps[0:Dm, :],
                                         mybir.ActivationFunctionType.Copy)
                else:
                    for k in range(Dout // 128):
                        tps = pst.tile([128, 128], f32, tag="tps")
                        nc.tensor.transpose(tps[:],
                                            hsb[:, k * 128:(k + 1) * 128],
                                            ident_sb)
                        nc.scalar.activation(hT[k][:, b * 128:(b + 1) * 128],
                                             tps[:],
                                             mybir.ActivationFunctionType.Copy)

            def mlp_block(b):
                """out rows of block b from h5 = hT[0][:, b] (128-wide)."""
                sl = slice(b * 128, (b + 1) * 128)
                p5t = ps.tile([128, DH], f32, tag="zps")
                p5 = p5t[0:MH, 0:128]
                nc.tensor.matmul(p5, M1_sb, hT[0][:, sl],
                                 start=True, stop=True)
                h5 = mp.tile([MH, 128], f16, tag="h5")
                nc.scalar.activation(h5[:], p5,
                                     mybir.ActivationFunctionType.Relu,
                                     bias=MB1_sb)
                p6t = psa.tile([128, DH], f32, tag="aps")
                p6 = p6t[0:MH, 0:128]
                nc.tensor.matmul(p6, M2_sb, h5[:], start=True, stop=True)
                h6 = mp.tile([MH, 128], f16, tag="h6")
                nc.scalar.activation(h6[:], p6,
                                     mybir.ActivationFunctionType.Relu,
                                     bias=MB2_sb)
                pot = pst.tile([128, 128], f32, tag="tps")
                po = pot[:, 0:NCLS]
                nc.tensor.matmul(po, h6[:], M3_sb, start=True, stop=True)
                osb = zp.tile([128, NCLS], f32, tag="osb")
                nc.vector.tensor_tensor(osb[:], po, MB3_sb,
                                        mybir.AluOpType.add)
                nc.sync.dma_start(out.ap()[sl, :], osb[:])

            # ---- software pipeline over layers ----
            # sub-collective k fires once the dense tiles covering its z rows
            # are emitted (the last sub fires after the loop)
            FIRE_AT = {sum(SUBBLK[:k + 1]) - 1: k for k in range(NSUB - 1)}
            # layer 1 dense alone (reads x), AG pieces fired asap
            for t in range(NB):
                dense_tile(1, t)
                if t in FIRE_AT:
                    fire_ag(1, FIRE_AT[t])
            fire_ag(1, NSUB - 1)

            # At each phase start the previous layer's last AllGather piece is
            # still in flight; emit the seg-0 gathers (ready data) of the
            # first K groups before any seg-1 gather so the gpsimd queue and
            # HBM stay busy through the collective window.
            K = 6
            for l in (2, 3, 4):
                # preload layer-3 dense weights into the now-idle parity
                # buffer (W1/W2/W4 were loaded at startup)
                if l == 3:
                    nc.sync.dma_start(W_sb[1], Wd[3].ap())
                ctxs = [agg_load(l - 1, g) for g in range(K)]
                for g in range(NG):
                    ctx = ctxs[g] if g < K else agg_load(l - 1, g)
                    for j in range(1, NPIECE):
                        agg_gather(ctx, j)
                    for k, b in enumerate(GROUPS[g]):
                        agg_compute(ctx, k)
                        dense_tile(l, b)
                        if b in FIRE_AT:
                            fire_ag(l, FIRE_AT[b])
                fire_ag(l, NSUB - 1)

            # final aggregation of layer 4 feeding the MLP head per block
            ctxs = [agg_load(4, g) for g in range(K)]
            for g in range(NG):
                ctx = ctxs[g] if g < K else agg_load(4, g)
                for j in range(1, NPIECE):
                    agg_gather(ctx, j)
                for k, b in enumerate(GROUPS[g]):
                    agg_compute(ctx, k)
                    mlp_block(b)

    nc.compile()
    return nc


# ---------------------------------------------------------------- entry point

def kernel(x, edge_index, W1, b1, W2, b2, W3, b3, W4, b4,
           M1, mb1, M2, mb2, M3, mb3):
    global LAST_RESULT
    x = np.asarray(x, np.float32)
    edge_index = np.asarray(edge_index)
    meta = _host_prep(x, edge_index)
    key = (tuple(meta["T_seg"].reshape(-1).tolist()), meta["TT"],
           tuple(meta["cmax"].reshape(-1).tolist()))
    if key not in _BUILD_CACHE:
        _BUILD_CACHE[key] = _build(key[0], key[1], key[2])
    nc = _BUILD_CACHE[key]

    W4p = np.zeros((DIN, 128), np.float32)
    W4p[:, :DE] = np.asarray(W4, np.float32)
    b4p = np.zeros(128, np.float32)
    b4p[:DE] = np.asarray(b4, np.float32)
    M1p = np.zeros((128, MH), np.float32)
    M1p[:DE] = np.asarray(M1, np.float32)

    # fold the fp8 z scales into weights/biases: W_l' = (c_l/c_{l-1}) W_l,
    # b_l' = c_l b_l  (c_0 = 1; c_4 = 1 so h4 and the MLP are unscaled)
    c = ZSCALE
    Wch = {1: _chunk_w(np.asarray(W1, np.float32) * c[1]),
           2: _chunk_w(np.asarray(W2, np.float32) * (c[2] / c[1])),
           3: _chunk_w(np.asarray(W3, np.float32) * (c[3] / c[2])),
           4: _chunk_w(W4p * (c[4] / c[3]))}
    Bb = {1: np.broadcast_to(np.asarray(b1, np.float32) * c[1], (128, DH)).copy(),
          2: np.broadcast_to(np.asarray(b2, np.float32) * c[2], (128, DH)).copy(),
          3: np.broadcast_to(np.asarray(b3, np.float32) * c[3], (128, DH)).copy(),
          4: np.broadcast_to(b4p * c[4], (128, 128)).copy()}

    common = {
        **{f"W{l}": Wch[l] for l in (1, 2, 3, 4)},
        **{f"B{l}": Bb[l] for l in (1, 2, 3, 4)},
        **{f"Br{l}": np.ascontiguousarray(Bb[l][0:1]).astype(np.float16)
           for l in (1, 2, 3, 4)},
        "M1p": M1p.astype(np.float16),
        "M2d": np.asarray(M2, np.float16),
        "M3d": np.asarray(M3, np.float16),
        "MB1": np.asarray(mb1, np.float32).reshape(MH, 1),
        "MB2": np.asarray(mb2, np.float32).reshape(MH, 1),
        "MB3b": np.broadcast_to(np.asarray(mb3, np.float32), (128, NCLS)).copy(),
        "ident_c": np.eye(128, dtype=np.float32),
        "iota_c": np.broadcast_to(np.arange(128, dtype=np.float16),
                                  (128, 128)).copy(),
    }

    in_maps = []
    for r in range(NCORES):
        rows = min(SHARD, max(0, N - r * SHARD))
        xp = np.zeros((SHARD, DIN), np.float32)
        xp[:rows] = x[r * SHARD:r * SHARD + rows]
        in_maps.append({
            **common,
            "xT": np.ascontiguousarray(xp.T).astype(np.float16),
            "gidx": meta["gidx"][r],
            "DCd": meta["DC"][r],
            "WWd": meta["WW"][r],
        })

    LAST_RESULT = bass_utils.run_bass_kernel_spmd(
        nc, in_maps, core_ids=list(range(NCORES)),
    )
    out = np.concatenate([LAST_RESULT.results[r]["out"] for r in range(NCORES)], 0)
    return np.ascontiguousarray(out[:N]).astype(np.float32)



# revision 69
# speedup vs baseline: 1.1006x; 1.1006x over previous
"""Trainium2 Bass kernel for nn_DomainGCN (4-layer GCN + MLP head), 8 NeuronCores.

Strategy (graph/data parallel, per sharding hint):
  - Nodes sharded contiguously: core r owns rows [r*6272, (r+1)*6272) (padded).
  - Reformulation: Agg(h@W) with per-edge weight dis[src]*dis[dst] and
    self-loops as a dedicated per-block self tile (weight dis[j]^2):
      z = h @ W            (dense, f32r matmuls, feature-major h in SBUF)
      z -> zdt -> AllGather (every core gets the full z, node-major, in DRAM)
      agg = S.T @ z[src]   (dma_gather of edge messages + one-hot-weighted
                            matmuls accumulating per 128-dst-node block in PSUM)
      h' = relu(agg + b)   (DVE add + ACT relu), PE-transposed back to
                            feature-major for the next dense matmul.
  - v3 (this session, 2811965ns -> ~1882535ns on HW):
    * SOFTWARE-PIPELINED emission: agg(l-1, b) and dense(l, b) interleaved
      per block; AllGather pieces of layer l fire mid-aggregation.
    * z payload fp8e4m3 for layers 1-3 (layer 3 stored scaled x16, layer 2
      x8 -- scales fold into W/b host-side via relu positive homogeneity,
      zero runtime cost); bf16 for layer 4 (256B dma_gather row minimum).
    * aggregation matmuls use DoubleRow perf mode on the fp8 layers (two
      128-edge tiles per instruction).
    * S one-hot tiles are built ON-CHIP (DVE: mask = iota==DC, S = mask*WW,
      batched per block with stride-0 broadcast APs) from [128, TT] f16
      metadata instead of streaming ~19MB/layer of host-built one-hots.
    * gather num_idxs statically trimmed to the worst-core true count
      (gbuf slots memset once so never-written tail rows stay finite).
    * deep prefetch: 6 gather buffers, K=6 groups preloaded at each phase
      boundary to ride out the in-flight tail AllGather piece.
    * asymmetric AG pieces [31, 18] blocks: the piece-1 segments then fit
      one 7-tile SWDGE call per block (3 gather calls/block, not 4) --
      GpSimd descriptor generation (994ns fixed/call) is the top bottleneck.
  - dma_gather indices are int16; AllGather runs as two piece collectives
    so gathered row ids stay < 32768.
  - Layer-4 aggregation feeds the MLP head block-by-block (short tail).
"""

import os
import math
import numpy as np

import concourse.bass as bass
import concourse.bacc as bacc
import concourse.mybir as mybir
import concourse.tile as tile
from concourse import bass_utils

# problem constants (hardcoded per task contract)
N, E = 50000, 800000
DIN = DH = 512
DE, MH, NCLS = 10, 64, 20
NCORES = 8
NB = 49                   # dst blocks of 128 nodes per core
SHARD = NB * 128          # 6272
NPAD = SHARD * NCORES     # 50176
KC = 4                    # 128-feature chunks of 512

f32 = mybir.dt.float32
f32r = mybir.dt.float32r
f16 = mybir.dt.float16
bf16 = mybir.dt.bfloat16
fp8 = mybir.dt.float8e4
i16 = mybir.dt.int16

# z-path dtype per layer (gather payload + AllGather + self tiles)
# Layers 1-3 fp8: z_l is stored scaled by ZSCALE[l] to center values in
# e4m3's range (z3 mean |z| ~0.05 underflows unscaled).  The scales fold
# into the dense weights/biases at zero runtime cost: relu is positively
# homogeneous, so h_l' = c_l*h_l propagates through W_{l+1}' =
# (c_{l+1}/c_l) * W_{l+1}.
ZDTL = {1: fp8, 2: fp8, 3: fp8, 4: bf16}
ZSCALE = {1: 1.0, 2: 8.0, 3: 16.0, 4: 1.0}
ZNPL = {l: mybir.dt.np(dt) for l, dt in ZDTL.items()}
DOUT = {1: DH, 2: DH, 3: DH, 4: 128}

# AllGather pieces (block-aligned): fired after dense tiles 30 / 48 of each
# layer.  (A 3-piece split was tried and regressed: collective time is
# floor-dominated at these sizes, so extra pieces cost more than the smaller
# exposed tail saves.)
# [31,18] asymmetric split: piece-1 segments then fit ONE 7-tile gather call
# per block (3 SWDGE calls/block instead of 4) -- the 994ns-fixed cost
# descriptor generation on GpSimd is the top bottleneck.  (This split
# regressed +53us in the old S-load/K=3 structure from DMA contention with
# the late piece-0 collective; DMA has since eased ~90%->~70% busy.)
PBLK = [31, 18]
PROW = [b * 128 for b in PBLK]            # rows per piece
PSTART = [0, PROW[0]]
NPIECE = 2

# Each piece ships as one sub-collective.  (Splitting piece-1 into two subs
# [31,9,9] -- smaller tail collective, same gather segments -- was tried and
# measured +25us: the per-collective floor outweighs the earlier landing.)
SUBBLK = [31, 18]
SUBPIECE = [0, 1]                          # which zf piece each sub writes
SUBROW = [b * 128 for b in SUBBLK]
SUBSTART = [0, 3968]                       # z_loc row range starts
SUBBASE = [0, 0]                           # dest row base within the piece zf
NSUB = 2

# dst blocks are processed one per group.  (Groups of 2 were tried to halve
# the 994ns-fixed-cost SWDGE gather calls and measured +89us WORSE: a
# block's compute then waits on the whole group's gathers -- per-block gbuf
# sync granularity is load-bearing for the gather/compute overlap.)
GROUPS = [(b,) for b in range(NB)]
NG = len(GROUPS)

LAST_RESULT = None        # BassKernelResults of the most recent run (for test.py)
_BUILD_CACHE = {}


# ---------------------------------------------------------------- host prep

def _host_prep(x, edge_index):
    src = edge_index[0].astype(np.int64)
    dst = edge_index[1].astype(np.int64)
    deg = np.bincount(dst, minlength=N).astype(np.float32) + 1.0
    dis = (1.0 / np.sqrt(deg)).astype(np.float32)

    sa, da = src, dst
    w = (dis[sa] * dis[da]).astype(np.float32)

    # Segments: src piece membership.  AllGather runs as NPIECE block-aligned
    # piece collectives, so the gathered tensors are zf[j] with rank-major
    # pieces; row ids stay < 32768 (int16-safe) with no extra split.
    r = da // SHARD
    b = (da % SHARD) // 128
    so = sa % SHARD
    k = np.searchsorted(np.array(SUBSTART[1:]), so, side="right")  # sub idx
    s = np.array(SUBPIECE)[k]                                      # piece idx
    row = (np.array(SUBBASE)[k] + (sa // SHARD) * np.array(SUBROW)[k]
           + (so - np.array(SUBSTART)[k]))         # row in zf[s]
    order = np.lexsort((row, s, b, r))
    row_s, da_s, w_s = row[order], da[order], w[order]
    key = (r * NB + b) * NPIECE + s
    ks = key[order]
    counts = np.bincount(ks, minlength=NCORES * NB * NPIECE)\
        .reshape(NCORES, NB, NPIECE)
    starts = np.zeros(NCORES * NB * NPIECE + 1, np.int64)
    np.cumsum(counts.reshape(-1), out=starts[1:])

    # uniform program structure: per (block, seg) tile count = max over cores
    T_seg = np.ceil(counts / 128.0).astype(np.int64).max(axis=0)  # [NB, NPIECE]
    TT = int(T_seg.sum()) + NB   # +1 self tile per block

    gidx, DC, WW = [], [], []
    for rr in range(NCORES):
        idx16 = np.zeros(TT * 128, np.int16)
        dcol = np.full(TT * 128, -1, np.int64)
        wcol = np.zeros(TT * 128, np.float32)
        cur = 0
        for B in GROUPS:
            # self tiles first (tile k of the group = block B[k]'s own
            # diag(dis^2); msg row p comes from local z row bb*128+p),
            # then per segment the blocks' edge tiles back to back.
            for bb in B:
                gnode = rr * SHARD + bb * 128 + np.arange(128)
                ok = gnode < N
                dcol[cur:cur + 128][ok] = np.arange(128)[ok]
                wcol[cur:cur + 128][ok] = (dis[gnode[ok]] ** 2)
                cur += 128
            for ss in range(NPIECE):
                for bb in B:
                    k = (rr * NB + bb) * NPIECE + ss
                    n = counts[rr, bb, ss]
                    lo, hi = starts[k], starts[k] + n
                    idx16[cur:cur + n] = row_s[lo:hi].astype(np.int16)
                    dcol[cur:cur + n] = da_s[lo:hi] - rr * SHARD - bb * 128
                    wcol[cur:cur + n] = w_s[lo:hi]
                    cur += int(T_seg[bb, ss]) * 128  # pads: idx 0, dst -1, w 0
        gidx.append(np.tile(idx16.reshape(-1, 16).T, (8, 1)))       # [128, TT*8]
        # S tiles are built ON-CHIP (DVE: iota==DC mask, then *WW) from the
        # per-edge (dst column, weight) metadata -- [128, TT] f16 each, edge
        # p of tile e at [p, e].  Pads have DC=-1 (mask never matches).
        DC.append(np.ascontiguousarray(
            dcol.reshape(TT, 128).T).astype(np.float16))
        WW.append(np.ascontiguousarray(
            wcol.reshape(TT, 128).T).astype(np.float16))

    return {
        "T_seg": T_seg, "TT": TT, "gidx": gidx, "DC": DC, "WW": WW,
        "cmax": counts.max(axis=0),   # true idx count per (block, seg)
    }


def _chunk_w(W):
    """[K, M] -> [128, (K//128)*M] with k-chunk c at cols [c*M, (c+1)*M)."""
    K, M = W.shape
    return np.ascontiguousarray(
        W.reshape(K // 128, 128, M).transpose(1, 0, 2).reshape(128, -1)
    ).astype(np.float16)


# ---------------------------------------------------------------- kernel build

def _build(T_seg_t, TT, cmax_t):
    T_seg = np.asarray(T_seg_t).reshape(NB, NPIECE)
    cmax = np.asarray(cmax_t).reshape(NB, NPIECE)
    # group tile counts: len(B) self tiles + per-seg per-block edge tiles
    GT = [len(B) + int(T_seg[list(B)].sum()) for B in GROUPS]
    GTMAX = max(GT)

    nc = bacc.Bacc("TRN2", target_bir_lowering=False, debug=False,
                   num_devices=NCORES, num_swdge_queues=4)

    dt_in = {}

    def din(name, shape, dt):
        dt_in[name] = nc.dram_tensor(name, shape, dt, kind="ExternalInput")
        return dt_in[name]

    xT = din("xT", [DIN, SHARD], f16)
    Wd = {l: din(f"W{l}", [128, KC * DOUT[l]], f16) for l in (1, 2, 3, 4)}
    Bd = {l: din(f"B{l}", [128, DOUT[l]], f32) for l in (1, 2, 3, 4)}
    Brd = {l: din(f"Br{l}", [1, DOUT[l]], f16) for l in (1, 2, 3, 4)}
    M1p = din("M1p", [128, MH], f16)
    M2d = din("M2d", [MH, MH], f16)
    M3d = din("M3d", [MH, NCLS], f16)
    MB1 = din("MB1", [MH, 1], f32)
    MB2 = din("MB2", [MH, 1], f32)
    MB3b = din("MB3b", [128, NCLS], f32)
    ident_c = din("ident_c", [128, 128], f32)
    gidx = din("gidx", [128, TT * 8], i16)
    DCd = din("DCd", [128, TT], f16)
    WWd = din("WWd", [128, TT], f16)
    iota_c = din("iota_c", [128, 128], f16)
    out = nc.dram_tensor("out", [SHARD, NCLS], f32, kind="ExternalOutput")

    # persistent SBUF (h and dense weights in fp16: 10-bit mantissa keeps the
    # dense path's error contribution ~0.05%/elem — bf16 was measured too
    # lossy — while halving the dominant SBUF footprint vs f32r)
    hT = [nc.alloc_sbuf_tensor(f"hT{k}", [128, SHARD], f16).ap() for k in range(KC)]
    W_sb = {p: nc.alloc_sbuf_tensor(f"W_sb{p}", [128, KC * DH], f16).ap()
            for p in (0, 1)}
    W4_sb = nc.alloc_sbuf_tensor("W4_sb", [128, KC * 128], f16).ap()
    ident_sb = nc.alloc_sbuf_tensor("ident_sb", [128, 128], f32).ap()
    DC_sb = nc.alloc_sbuf_tensor("DC_sb", [128, TT], f16).ap()
    WW_sb = nc.alloc_sbuf_tensor("WW_sb", [128, TT], f16).ap()
    iota_sb = nc.alloc_sbuf_tensor("iota_sb", [128, 128], f16).ap()
    Bb_sb = {l: nc.alloc_sbuf_tensor(f"Bb{l}", [128, DOUT[l]], f32).ap()
             for l in (1, 2, 3, 4)}
    Brow_sb = {l: nc.alloc_sbuf_tensor(f"Brow{l}", [1, DOUT[l]], f16).ap()
               for l in (1, 2, 3, 4)}
    ones_sb = nc.alloc_sbuf_tensor("ones_sb", [1, 128], f16).ap()
    M1_sb = nc.alloc_sbuf_tensor("M1_sb", [128, MH], f16).ap()
    M2_sb = nc.alloc_sbuf_tensor("M2_sb", [MH, MH], f16).ap()
    M3_sb = nc.alloc_sbuf_tensor("M3_sb", [MH, NCLS], f16).ap()
    MB1_sb = nc.alloc_sbuf_tensor("MB1_sb", [MH, 1], f32).ap()
    MB2_sb = nc.alloc_sbuf_tensor("MB2_sb", [MH, 1], f32).ap()
    MB3_sb = nc.alloc_sbuf_tensor("MB3_sb", [128, NCLS], f32).ap()

    # DRAM z buffers, one set per layer (race-free pipelining across layers)
    z_loc = {l: nc.dram_tensor(f"zloc{l}", [SHARD, DOUT[l]], ZDTL[l],
                               kind="Internal").ap() for l in (1, 2, 3, 4)}


    zf = {l: tuple(nc.dram_tensor(f"zf{l}_{j}", [NCORES * PROW[j], DOUT[l]],
                                  ZDTL[l], kind="Internal",
                                  addr_space="Shared").ap()
                   for j in range(NPIECE))
          for l in (1, 2, 3, 4)}

    rg = [list(range(NCORES))]

    # column offsets of each group in gidx/DC/WW, and within-group tile
    # offsets: [selfs][seg0: blocks][seg1: blocks]
    gcol_of = np.concatenate([[0], np.cumsum(GT)]).astype(int)

    def seg_off(g, j):
        """Offset of seg j's tile run within group g (all blocks)."""
        B = GROUPS[g]
        return len(B) + int(T_seg[list(B), :j].sum())

    def blk_off(g, j, k):
        """Offset of block B[k]'s seg-j tile run within group g."""
        B = GROUPS[g]
        return seg_off(g, j) + int(T_seg[list(B[:k]), j].sum())

    with tile.TileContext(nc) as tc:
        with (
            tc.tile_pool(name="meta", bufs=8) as meta,
            tc.tile_pool(name="gp", bufs=6) as gp,
            tc.tile_pool(name="sp", bufs=6) as sp,
            tc.tile_pool(name="mk", bufs=2) as mk,
            tc.tile_pool(name="zp", bufs=3) as zp,
            tc.tile_pool(name="hp", bufs=3) as hp,
            tc.tile_pool(name="mp", bufs=3) as mp,
            tc.tile_pool(name="ps", bufs=2, space="PSUM") as ps,
            tc.tile_pool(name="psa", bufs=3, space="PSUM") as psa,
            tc.tile_pool(name="pst", bufs=2, space="PSUM") as pst,
        ):
            # ---- constant / weight loads (hT in column chunks so dense L1
            # can start early)
            # startup loads split across both HWDGE rings (scalar is otherwise
            # idle until the first agg phase) so dense L1 and the first
            # AllGather fire sooner
            nc.scalar.dma_start(W_sb[1], Wd[1].ap())
            nc.sync.dma_start(ident_sb, ident_c.ap())
            nc.scalar.dma_start(DC_sb, DCd.ap())
            nc.scalar.dma_start(WW_sb, WWd.ap())
            nc.scalar.dma_start(iota_sb, iota_c.ap())
            CCH = SHARD // 4
            for c in range(4):
                for k in range(KC):
                    eng = nc.sync if k % 2 == 0 else nc.scalar
                    eng.dma_start(
                        hT[k][:, c * CCH:(c + 1) * CCH],
                        xT[k * 128:(k + 1) * 128, c * CCH:(c + 1) * CCH])
            nc.sync.dma_start(W_sb[0], Wd[2].ap())
            nc.sync.dma_start(W4_sb, Wd[4].ap())
            for l in (1, 2, 3, 4):
                nc.sync.dma_start(Bb_sb[l], Bd[l].ap())
                nc.sync.dma_start(Brow_sb[l], Brd[l].ap())
            nc.vector.memset(ones_sb, 1.0)
            nc.sync.dma_start(M1_sb, M1p.ap())
            nc.sync.dma_start(M2_sb, M2d.ap())
            nc.sync.dma_start(M3_sb, M3d.ap())
            nc.sync.dma_start(MB1_sb, MB1.ap())
            nc.sync.dma_start(MB2_sb, MB2.ap())
            nc.sync.dma_start(MB3_sb, MB3b.ap())

            # SWDGE queue load balancing (greedy by tile count)
            qload = [0, 0, 0, 0]

            # zero the gather-buffer slots once: trimmed tail rows are never
            # gathered, and uninitialized SBUF could hold fp8 NaN patterns
            for _ in range(6):
                zt = gp.tile([128, GTMAX, DH], fp8, tag="gbuf", name="gbuf8")
                nc.vector.memset(zt[:], 0)

            def gbuf_tile(l):
                # All layers share one pool slot family (same bytes per slot):
                #   l=1..3: fp8  [128, GTMAX, 512] -> group tiles at 512B
                #   l=4:    bf16 [128, 2*GTMAX, 128] -> group tiles at 256B
                if l <= 3:
                    return gp.tile([128, GTMAX, DH], fp8, tag="gbuf",
                                   name="gbuf8")
                return gp.tile([128, 2 * GTMAX, 128], bf16, tag="gbuf",
                               name="gbuf4")

            def dense_tile(l, t):
                """z_l[t] = hT[:, t] @ W_l -> zdt -> DRAM z_loc."""
                Dout = DOUT[l]
                wsb = W_sb[l % 2] if l < 4 else W4_sb
                zps = ps.tile([128, DH], f32, tag="zps")
                for k in range(KC):
                    nc.tensor.matmul(
                        zps[:, 0:Dout],
                        hT[k][:, t * 128:(t + 1) * 128],
                        wsb[:, k * Dout:(k + 1) * Dout],
                        start=(k == 0), stop=(k == KC - 1),
                    )
                zsb = zp.tile([128, Dout], ZDTL[l], tag=f"zsb_{l}")
                nc.scalar.activation(zsb[:], zps[:, 0:Dout],
                                     mybir.ActivationFunctionType.Copy)
                nc.sync.dma_start(z_loc[l][t * 128:(t + 1) * 128, :], zsb[:])

            def fire_ag(l, k):
                """Fire sub-collective k (writes a row slice of its piece)."""
                j = SUBPIECE[k]
                nc.gpsimd.collective_compute(
                    "AllGather", mybir.AluOpType.bypass,
                    replica_groups=rg,
                    ins=[z_loc[l][SUBSTART[k]:SUBSTART[k] + SUBROW[k], :]],
                    outs=[zf[l][j][SUBBASE[k]:SUBBASE[k]
                                   + NCORES * SUBROW[k], :]],
                )

            def agg_load(l, g):
                """gbuf alloc + idx/S/self loads + seg-0 gathers for group g."""
                Dout = DOUT[l]
                B = GROUPS[g]
                TG = GT[g]
                col = int(gcol_of[g])
                gbuf = gbuf_tile(l)
                idx_sb = meta.tile([128, TG * 8], i16, tag="idx")
                nc.sync.dma_start(idx_sb[:], gidx.ap()[:, col * 8:(col + TG) * 8])
                # Build S on-chip (saves ~19MB of HBM one-hot traffic per
                # layer): mask = (iota == DC), S = mask * WW, batched over
                # the whole group's TG tiles with stride-0 broadcast APs.
                # (Caching built S in per-block DRAM tensors for layers 2-3
                # was tried twice and regressed both times (+103us, +35us):
                # DVE runs in parallel slack here -- GatherGen+DMA bind, and
                # the cache traffic re-pressures DMA.)
                sdt = fp8 if l <= 3 else bf16
                S_sb = sp.tile([128, TG, 128], sdt, tag="S8" if l <= 3 else "S")
                msk = mk.tile([128, TG, 128], f16, tag="msk")
                i3 = iota_sb.rearrange("p (a d) -> p a d", a=1)      # [128,1,128]
                d3 = DC_sb[:, col:col + TG]\
                    .rearrange("p (t a) -> p t a", a=1)              # [128,TG,1]
                ib, db = bass.broadcast_tensor_aps(i3, d3)
                nc.vector.tensor_tensor(msk[:], ib, db,
                                        mybir.AluOpType.is_equal)
                w3 = WW_sb[:, col:col + TG]\
                    .rearrange("p (t a) -> p t a", a=1)              # [128,TG,1]
                mb_, wb = bass.broadcast_tensor_aps(msk[:], w3)
                nc.vector.tensor_tensor(S_sb[:], mb_, wb,
                                        mybir.AluOpType.mult)

                # self tiles: one contiguous copy of the group's own z rows
                nb = len(B)
                nc.sync.dma_start(gbuf[:, 0:nb, :],
                                  z_loc[l][B[0] * 128:(B[0] + nb) * 128, :]
                                  .rearrange("(a p) d -> p a d", a=nb))
                ctx = (l, g, Dout, gbuf, idx_sb, S_sb)
                agg_gather(ctx, 0)
                return ctx

            def agg_gather(ctx, j):
                """Emit the seg-j gathers for group g (all blocks)."""
                l, g, Dout, gbuf, idx_sb, S_sb = ctx
                off = seg_off(g, j)
                # SWDGE descriptor ring holds ~1024 descs/queue (ucode
                # constant — it does NOT grow with dynamic_dma_scratch_size;
                # bigger calls hang the device).  Cap at 7 tiles (896 idxs).
                # num_idxs is trimmed to the worst-core true count (the tail
                # pads are never gathered; their S rows are zero and gbuf
                # slots are memset once at startup, so stale rows are finite).
                left = int(T_seg[list(GROUPS[g]), j].sum())
                rows = int(sum(cmax[b, j] for b in GROUPS[g]))
                while left > 0:
                    sub = min(7, left)
                    n = min(sub * 128, rows)
                    nt = (n + 127) // 128
                    qn = qload.index(min(qload))
                    qload[qn] += sub
                    nc.gpsimd.dma_gather(
                        gbuf[:, off:off + nt, :],
                        zf[l][j],
                        idx_sb[:, off * 8:(off + nt) * 8],
                        num_idxs=n, num_idxs_reg=n, elem_size=Dout,
                        queue_num=qn,
                    )
                    off += sub
                    left -= sub
                    rows -= n

            def agg_compute(ctx, k):
                """S.T @ gbuf accumulation + epilogue for block B[k] of g."""
                l, g, Dout, gbuf, idx_sb, S_sb = ctx
                b = GROUPS[g][k]
                # tile runs of this block: [self] + per-seg slices; merge
                # adjacent contiguous runs so DoubleRow pairs span them
                runs = [(k, 1)] + [
                    (blk_off(g, j, k), int(T_seg[b, j]))
                    for j in range(NPIECE)
                ]
                merged = [runs[0]]
                for off, cnt in runs[1:]:
                    po, pc = merged[-1]
                    if off == po + pc:
                        merged[-1] = (po, pc + cnt)
                    else:
                        merged.append((off, cnt))
                runs = merged
                # (Narrowing layer 4's matmuls to the 16 real feature cols
                # was tried and measured +130us: a 16-cycle stream can't hide
                # the 128-col LDWEIGHTS, so every matmul goes LDW-bound.)
                Dm = Dout
                aps = psa.tile([128, DH], f32, tag="aps")
                first = True
                for off, cnt in runs:
                    e = off
                    while e < off + cnt:
                        if l <= 3 and e + 1 < off + cnt:
                            # fp8: DoubleRow packs two 128-edge tiles per
                            # instruction (contracts 256 edges)
                            nc.tensor.matmul(
                                aps[:, 0:Dm],
                                S_sb[:, e:e + 2, :], gbuf[:, e:e + 2, :],
                                start=first, stop=False,
                                perf_mode=mybir.MatmulPerfMode.DoubleRow,
                            )
                            e += 2
                        else:
                            nc.tensor.matmul(
                                aps[:, 0:Dm], S_sb[:, e, :],
                                gbuf[:, e, 0:Dm],
                                start=first, stop=False,
                            )
                            e += 1
                        first = False
                # bias folded into the accumulation as a rank-1 matmul
                # (ones.T @ bias_row) so the relu can read PSUM directly
                nc.tensor.matmul(
                    aps[:, 0:Dm], ones_sb[0:1, 0:128],
                    Brow_sb[l][0:1, 0:Dm],
                    start=False, stop=True,
                )

                # epilogue: h = relu(agg + b) straight from PSUM (bias is
                # already accumulated); transpose back to feature-major
                hsb = hp.tile([128, Dm], f32, tag="hsb")
                nc.scalar.activation(hsb[:], aps[:, 0:Dm],
                                     mybir.ActivationFunctionType.Relu)
                for k in range(Dout // 128):
                    tps = pst.tile([128, 128], f32, tag="tps")
                    nc.tensor.transpose(tps[:],
                                        hsb[:, k * 128:(k + 1) * 128],
                                        ident_sb)
                    nc.scalar.activation(hT[k][:, b * 128:(b + 1) * 128],
                                         tps[:],
                                         mybir.ActivationFunctionType.Copy)

            def mlp_block(b):
                """out rows of block b from h5 = hT[0][:, b] (128-wide)."""
                sl = slice(b * 128, (b + 1) * 128)
                p5t = ps.tile([128, DH], f32, tag="zps")
                p5 = p5t[0:MH, 0:128]
                nc.tensor.matmul(p5, M1_sb, hT[0][:, sl],
                                 start=True, stop=True)
                h5 = mp.tile([MH, 128], f16, tag="h5")
                nc.scalar.activation(h5[:], p5,
                                     mybir.ActivationFunctionType.Relu,
                                     bias=MB1_sb)
                p6t = psa.tile([128, DH], f32, tag="aps")
                p6 = p6t[0:MH, 0:128]
                nc.tensor.matmul(p6, M2_sb, h5[:], start=True, stop=True)
                h6 = mp.tile([MH, 128], f16, tag="h6")
                nc.scalar.activation(h6[:], p6,
                                     mybir.ActivationFunctionType.Relu,
                                     bias=MB2_sb)
                pot = pst.tile([128, 128], f32, tag="tps")
                po = pot[:, 0:NCLS]
                nc.tensor.matmul(po, h6[:], M3_sb, start=True, stop=True)
                osb = zp.tile([128, NCLS], f32, tag="osb")
                nc.vector.tensor_tensor(osb[:], po, MB3_sb,
                                        mybir.AluOpType.add)
                nc.sync.dma_start(out.ap()[sl, :], osb[:])

            # ---- software pipeline over layers ----
            # sub-collective k fires once the dense tiles covering its z rows
            # are emitted (the last sub fires after the loop)
            FIRE_AT = {sum(SUBBLK[:k + 1]) - 1: k for k in range(NSUB - 1)}
            # layer 1 dense alone (reads x), AG pieces fired asap
            for t in range(NB):
                dense_tile(1, t)
                if t in FIRE_AT:
                    fire_ag(1, FIRE_AT[t])
            fire_ag(1, NSUB - 1)

            # At each phase start the previous layer's last AllGather piece is
            # still in flight; emit the seg-0 gathers (ready data) of the
            # first K groups before any seg-1 gather so the gpsimd queue and
            # HBM stay busy through the collective window.
            K = 6
            for l in (2, 3, 4):
                # preload layer-3 dense weights into the now-idle parity
                # buffer (W1/W2/W4 were loaded at startup)
                if l == 3:
                    nc.sync.dma_start(W_sb[1], Wd[3].ap())
                ctxs = [agg_load(l - 1, g) for g in range(K)]
                for g in range(NG):
                    ctx = ctxs[g] if g < K else agg_load(l - 1, g)
                    for j in range(1, NPIECE):
                        agg_gather(ctx, j)
                    for k, b in enumerate(GROUPS[g]):
                        agg_compute(ctx, k)
                        dense_tile(l, b)
                        if b in FIRE_AT:
                            fire_ag(l, FIRE_AT[b])
                fire_ag(l, NSUB - 1)

            # final aggregation of layer 4 feeding the MLP head per block
            ctxs = [agg_load(4, g) for g in range(K)]
            for g in range(NG):
                ctx = ctxs[g] if g < K else agg_load(4, g)
                for j in range(1, NPIECE):
                    agg_gather(ctx, j)
                for k, b in enumerate(GROUPS[g]):
                    agg_compute(ctx, k)
                    mlp_block(b)

    nc.compile()
    return nc


# ---------------------------------------------------------------- entry point

def kernel(x, edge_index, W1, b1, W2, b2, W3, b3, W4, b4,
           M1, mb1, M2, mb2, M3, mb3):
    global LAST_RESULT
    x = np.asarray(x, np.float32)
    edge_index = np.asarray(edge_index)
    meta = _host_prep(x, edge_index)
    key = (tuple(meta["T_seg"].reshape(-1).tolist()), meta["TT"],
           tuple(meta["cmax"].reshape(-1).tolist()))
    if key not in _BUILD_CACHE:
        _BUILD_CACHE[key] = _build(key[0], key[1], key[2])
    nc = _BUILD_CACHE[key]

    W4p = np.zeros((DIN, 128), np.float32)
    W4p[:, :DE] = np.asarray(W4, np.float32)
    b4p = np.zeros(128, np.float32)
    b4p[:DE] = np.asarray(b4, np.float32)
    M1p = np.zeros((128, MH), np.float32)
    M1p[:DE] = np.asarray(M1, np.float32)

    # fold the fp8 z scales into weights/biases: W_l' = (c_l/c_{l-1}) W_l,
    # b_l' = c_l b_l  (c_0 = 1; c_4 = 1 so h4 and the MLP are unscaled)
    c = ZSCALE
    Wch = {1: _chunk_w(np.asarray(W1, np.float32) * c[1]),
           2: _chunk_w(np.asarray(W2, np.float32) * (c[2] / c[1])),
           3: _chunk_w(np.asarray(W3, np.float32) * (c[3] / c[2])),
           4: _chunk_w(W4p * (c[4] / c[3]))}
    Bb = {1: np.broadcast_to(np.asarray(b1, np.float32) * c[1], (128, DH)).copy(),
          2: np.broadcast_to(np.asarray(b2, np.float32) * c[2], (128, DH)).copy(),
          3: np.broadcast_to(np.asarray(b3, np.float32) * c[3], (128, DH)).copy(),
          4: np.broadcast_to(b4p * c[4], (128, 128)).copy()}

    common = {
        **{f"W{l}": Wch[l] for l in (1, 2, 3, 4)},
        **{f"B{l}": Bb[l] for l in (1, 2, 3, 4)},
        **{f"Br{l}": np.ascontiguousarray(Bb[l][0:1]).astype(np.float16)
           for l in (1, 2, 3, 4)},
        "M1p": M1p.astype(np.float16),
        "M2d": np.asarray(M2, np.float16),
        "M3d": np.asarray(M3, np.float16),
        "MB1": np.asarray(mb1, np.float32).reshape(MH, 1),
        "MB2": np.asarray(mb2, np.float32).reshape(MH, 1),
        "MB3b": np.broadcast_to(np.asarray(mb3, np.float32), (128, NCLS)).copy(),
        "ident_c": np.eye(128, dtype=np.float32),
        "iota_c": np.broadcast_to(np.arange(128, dtype=np.float16),
                                  (128, 128)).copy(),
    }

    in_maps = []
    for r in range(NCORES):
        rows = min(SHARD, max(0, N - r * SHARD))
        xp = np.zeros((SHARD, DIN), np.float32)
        xp[:rows] = x[r * SHARD:r * SHARD + rows]
        in_maps.append({
            **common,
            "xT": np.ascontiguousarray(xp.T).astype(np.float16),
            "gidx": meta["gidx"][r],
            "DCd": meta["DC"][r],
            "WWd": meta["WW"][r],
        })

    LAST_RESULT = bass_utils.run_bass_kernel_spmd(
        nc, in_maps, core_ids=list(range(NCORES)),
    )
    out = np.concatenate([LAST_RESULT.results[r]["out"] for r in range(NCORES)], 0)
    return np.ascontiguousarray(out[:N]).astype(np.float32)



# revision 70
# speedup vs baseline: 1.1088x; 1.0075x over previous
"""Trainium2 Bass kernel for nn_DomainGCN (4-layer GCN + MLP head), 8 NeuronCores.

Strategy (graph/data parallel, per sharding hint):
  - Nodes sharded contiguously: core r owns rows [r*6272, (r+1)*6272) (padded).
  - Reformulation: Agg(h@W) with per-edge weight dis[src]*dis[dst] and
    self-loops as a dedicated per-block self tile (weight dis[j]^2):
      z = h @ W            (dense, f32r matmuls, feature-major h in SBUF)
      z -> zdt -> AllGather (every core gets the full z, node-major, in DRAM)
      agg = S.T @ z[src]   (dma_gather of edge messages + one-hot-weighted
                            matmuls accumulating per 128-dst-node block in PSUM)
      h' = relu(agg + b)   (DVE add + ACT relu), PE-transposed back to
                            feature-major for the next dense matmul.
  - v3 (this session, 2811965ns -> ~1882535ns on HW):
    * SOFTWARE-PIPELINED emission: agg(l-1, b) and dense(l, b) interleaved
      per block; AllGather pieces of layer l fire mid-aggregation.
    * z payload fp8e4m3 for layers 1-3 (layer 3 stored scaled x16, layer 2
      x8 -- scales fold into W/b host-side via relu positive homogeneity,
      zero runtime cost); bf16 for layer 4 (256B dma_gather row minimum).
    * aggregation matmuls use DoubleRow perf mode on the fp8 layers (two
      128-edge tiles per instruction).
    * S one-hot tiles are built ON-CHIP (DVE: mask = iota==DC, S = mask*WW,
      batched per block with stride-0 broadcast APs) from [128, TT] f16
      metadata instead of streaming ~19MB/layer of host-built one-hots.
    * gather num_idxs statically trimmed to the worst-core true count
      (gbuf slots memset once so never-written tail rows stay finite).
    * deep prefetch: 6 gather buffers, K=6 groups preloaded at each phase
      boundary to ride out the in-flight tail AllGather piece.
    * asymmetric AG pieces [31, 18] blocks: the piece-1 segments then fit
      one 7-tile SWDGE call per block (3 gather calls/block, not 4) --
      GpSimd descriptor generation (994ns fixed/call) is the top bottleneck.
  - dma_gather indices are int16; AllGather runs as two piece collectives
    so gathered row ids stay < 32768.
  - Layer-4 aggregation feeds the MLP head block-by-block (short tail).
"""

import os
import math
import numpy as np

import concourse.bass as bass
import concourse.bacc as bacc
import concourse.mybir as mybir
import concourse.tile as tile
from concourse import bass_utils

# problem constants (hardcoded per task contract)
N, E = 50000, 800000
DIN = DH = 512
DE, MH, NCLS = 10, 64, 20
NCORES = 8
NB = 49                   # dst blocks of 128 nodes per core
SHARD = NB * 128          # 6272
NPAD = SHARD * NCORES     # 50176
KC = 4                    # 128-feature chunks of 512

f32 = mybir.dt.float32
f32r = mybir.dt.float32r
f16 = mybir.dt.float16
bf16 = mybir.dt.bfloat16
fp8 = mybir.dt.float8e4
i16 = mybir.dt.int16

# z-path dtype per layer (gather payload + AllGather + self tiles)
# Layers 1-3 fp8: z_l is stored scaled by ZSCALE[l] to center values in
# e4m3's range (z3 mean |z| ~0.05 underflows unscaled).  The scales fold
# into the dense weights/biases at zero runtime cost: relu is positively
# homogeneous, so h_l' = c_l*h_l propagates through W_{l+1}' =
# (c_{l+1}/c_l) * W_{l+1}.
ZDTL = {1: fp8, 2: fp8, 3: fp8, 4: bf16}
ZSCALE = {1: 1.0, 2: 8.0, 3: 16.0, 4: 1.0}
ZNPL = {l: mybir.dt.np(dt) for l, dt in ZDTL.items()}
DOUT = {1: DH, 2: DH, 3: DH, 4: 128}

# AllGather pieces (block-aligned): fired after dense tiles 30 / 48 of each
# layer.  (A 3-piece split was tried and regressed: collective time is
# floor-dominated at these sizes, so extra pieces cost more than the smaller
# exposed tail saves.)
# [31,18] asymmetric split: piece-1 segments then fit ONE 7-tile gather call
# per block (3 SWDGE calls/block instead of 4) -- the 994ns-fixed cost
# descriptor generation on GpSimd is the top bottleneck.  (This split
# regressed +53us in the old S-load/K=3 structure from DMA contention with
# the late piece-0 collective; DMA has since eased ~90%->~70% busy.)
PBLK = [31, 18]
PROW = [b * 128 for b in PBLK]            # rows per piece
PSTART = [0, PROW[0]]
NPIECE = 2

# Each piece ships as one sub-collective.  (Splitting piece-1 into two subs
# [31,9,9] -- smaller tail collective, same gather segments -- was tried and
# measured +25us: the per-collective floor outweighs the earlier landing.)
SUBBLK = [31, 18]
SUBPIECE = [0, 1]                          # which zf piece each sub writes
SUBROW = [b * 128 for b in SUBBLK]
SUBSTART = [0, 3968]                       # z_loc row range starts
SUBBASE = [0, 0]                           # dest row base within the piece zf
NSUB = 2

# dst blocks are processed one per group.  (Groups of 2 were tried to halve
# the 994ns-fixed-cost SWDGE gather calls and measured +89us WORSE: a
# block's compute then waits on the whole group's gathers -- per-block gbuf
# sync granularity is load-bearing for the gather/compute overlap.)
GROUPS = [(b,) for b in range(NB)]
NG = len(GROUPS)

LAST_RESULT = None        # BassKernelResults of the most recent run (for test.py)
_BUILD_CACHE = {}


# ---------------------------------------------------------------- host prep

def _host_prep(x, edge_index):
    src = edge_index[0].astype(np.int64)
    dst = edge_index[1].astype(np.int64)
    deg = np.bincount(dst, minlength=N).astype(np.float32) + 1.0
    dis = (1.0 / np.sqrt(deg)).astype(np.float32)

    sa, da = src, dst
    w = (dis[sa] * dis[da]).astype(np.float32)

    # Segments: src piece membership.  AllGather runs as NPIECE block-aligned
    # piece collectives, so the gathered tensors are zf[j] with rank-major
    # pieces; row ids stay < 32768 (int16-safe) with no extra split.
    r = da // SHARD
    b = (da % SHARD) // 128
    so = sa % SHARD
    k = np.searchsorted(np.array(SUBSTART[1:]), so, side="right")  # sub idx
    s = np.array(SUBPIECE)[k]                                      # piece idx
    row = (np.array(SUBBASE)[k] + (sa // SHARD) * np.array(SUBROW)[k]
           + (so - np.array(SUBSTART)[k]))         # row in zf[s]
    order = np.lexsort((row, s, b, r))
    row_s, da_s, w_s = row[order], da[order], w[order]
    key = (r * NB + b) * NPIECE + s
    ks = key[order]
    counts = np.bincount(ks, minlength=NCORES * NB * NPIECE)\
        .reshape(NCORES, NB, NPIECE)
    starts = np.zeros(NCORES * NB * NPIECE + 1, np.int64)
    np.cumsum(counts.reshape(-1), out=starts[1:])

    # uniform program structure: per (block, seg) tile count = max over cores
    T_seg = np.ceil(counts / 128.0).astype(np.int64).max(axis=0)  # [NB, NPIECE]
    TT = int(T_seg.sum()) + NB   # +1 self tile per block

    gidx, DC, WW = [], [], []
    for rr in range(NCORES):
        idx16 = np.zeros(TT * 128, np.int16)
        dcol = np.full(TT * 128, -1, np.int64)
        wcol = np.zeros(TT * 128, np.float32)
        cur = 0
        for B in GROUPS:
            # self tiles first (tile k of the group = block B[k]'s own
            # diag(dis^2); msg row p comes from local z row bb*128+p),
            # then per segment the blocks' edge tiles back to back.
            for bb in B:
                gnode = rr * SHARD + bb * 128 + np.arange(128)
                ok = gnode < N
                dcol[cur:cur + 128][ok] = np.arange(128)[ok]
                wcol[cur:cur + 128][ok] = (dis[gnode[ok]] ** 2)
                cur += 128
            for ss in range(NPIECE):
                for bb in B:
                    k = (rr * NB + bb) * NPIECE + ss
                    n = counts[rr, bb, ss]
                    lo, hi = starts[k], starts[k] + n
                    idx16[cur:cur + n] = row_s[lo:hi].astype(np.int16)
                    dcol[cur:cur + n] = da_s[lo:hi] - rr * SHARD - bb * 128
                    wcol[cur:cur + n] = w_s[lo:hi]
                    cur += int(T_seg[bb, ss]) * 128  # pads: idx 0, dst -1, w 0
        gidx.append(np.tile(idx16.reshape(-1, 16).T, (8, 1)))       # [128, TT*8]
        # S tiles are built ON-CHIP (DVE: iota==DC mask, then *WW) from the
        # per-edge (dst column, weight) metadata -- [128, TT] f16 each, edge
        # p of tile e at [p, e].  Pads have DC=-1 (mask never matches).
        DC.append(np.ascontiguousarray(
            dcol.reshape(TT, 128).T).astype(np.float16))
        WW.append(np.ascontiguousarray(
            wcol.reshape(TT, 128).T).astype(np.float16))

    return {
        "T_seg": T_seg, "TT": TT, "gidx": gidx, "DC": DC, "WW": WW,
        "cmax": counts.max(axis=0),   # true idx count per (block, seg)
    }


def _chunk_w(W):
    """[K, M] -> [128, (K//128)*M] with k-chunk c at cols [c*M, (c+1)*M)."""
    K, M = W.shape
    return np.ascontiguousarray(
        W.reshape(K // 128, 128, M).transpose(1, 0, 2).reshape(128, -1)
    ).astype(np.float16)


# ---------------------------------------------------------------- kernel build

def _build(T_seg_t, TT, cmax_t):
    T_seg = np.asarray(T_seg_t).reshape(NB, NPIECE)
    cmax = np.asarray(cmax_t).reshape(NB, NPIECE)
    # group tile counts: len(B) self tiles + per-seg per-block edge tiles
    GT = [len(B) + int(T_seg[list(B)].sum()) for B in GROUPS]
    GTMAX = max(GT)

    nc = bacc.Bacc("TRN2", target_bir_lowering=False, debug=False,
                   num_devices=NCORES, num_swdge_queues=4)

    dt_in = {}

    def din(name, shape, dt):
        dt_in[name] = nc.dram_tensor(name, shape, dt, kind="ExternalInput")
        return dt_in[name]

    xT = din("xT", [DIN, SHARD], f16)
    Wd = {l: din(f"W{l}", [128, KC * DOUT[l]], f16) for l in (1, 2, 3, 4)}
    Bd = {l: din(f"B{l}", [128, DOUT[l]], f32) for l in (1, 2, 3, 4)}
    Brd = {l: din(f"Br{l}", [1, DOUT[l]], f16) for l in (1, 2, 3, 4)}
    M1p = din("M1p", [128, MH], f16)
    M2d = din("M2d", [MH, MH], f16)
    M3d = din("M3d", [MH, NCLS], f16)
    MB1 = din("MB1", [MH, 1], f32)
    MB2 = din("MB2", [MH, 1], f32)
    MB3b = din("MB3b", [128, NCLS], f32)
    ident_c = din("ident_c", [128, 128], f32)
    gidx = din("gidx", [128, TT * 8], i16)
    DCd = din("DCd", [128, TT], f16)
    WWd = din("WWd", [128, TT], f16)
    iota_c = din("iota_c", [128, 128], f16)
    out = nc.dram_tensor("out", [SHARD, NCLS], f32, kind="ExternalOutput")

    # persistent SBUF (h and dense weights in fp16: 10-bit mantissa keeps the
    # dense path's error contribution ~0.05%/elem — bf16 was measured too
    # lossy — while halving the dominant SBUF footprint vs f32r)
    hT = [nc.alloc_sbuf_tensor(f"hT{k}", [128, SHARD], f16).ap() for k in range(KC)]
    W_sb = {p: nc.alloc_sbuf_tensor(f"W_sb{p}", [128, KC * DH], f16).ap()
            for p in (0, 1)}
    W4_sb = nc.alloc_sbuf_tensor("W4_sb", [128, KC * 128], f16).ap()
    ident_sb = nc.alloc_sbuf_tensor("ident_sb", [128, 128], f32).ap()
    DC_sb = nc.alloc_sbuf_tensor("DC_sb", [128, TT], f16).ap()
    WW_sb = nc.alloc_sbuf_tensor("WW_sb", [128, TT], f16).ap()
    iota_sb = nc.alloc_sbuf_tensor("iota_sb", [128, 128], f16).ap()
    Bb_sb = {l: nc.alloc_sbuf_tensor(f"Bb{l}", [128, DOUT[l]], f32).ap()
             for l in (1, 2, 3, 4)}
    Brow_sb = {l: nc.alloc_sbuf_tensor(f"Brow{l}", [1, DOUT[l]], f16).ap()
               for l in (1, 2, 3, 4)}
    ones_sb = nc.alloc_sbuf_tensor("ones_sb", [1, 128], f16).ap()
    M1_sb = nc.alloc_sbuf_tensor("M1_sb", [128, MH], f16).ap()
    M2_sb = nc.alloc_sbuf_tensor("M2_sb", [MH, MH], f16).ap()
    M3_sb = nc.alloc_sbuf_tensor("M3_sb", [MH, NCLS], f16).ap()
    MB1_sb = nc.alloc_sbuf_tensor("MB1_sb", [MH, 1], f32).ap()
    MB2_sb = nc.alloc_sbuf_tensor("MB2_sb", [MH, 1], f32).ap()
    MB3_sb = nc.alloc_sbuf_tensor("MB3_sb", [128, NCLS], f32).ap()

    # DRAM z buffers, one set per layer (race-free pipelining across layers)
    z_loc = {l: nc.dram_tensor(f"zloc{l}", [SHARD, DOUT[l]], ZDTL[l],
                               kind="Internal").ap() for l in (1, 2, 3, 4)}


    zf = {l: tuple(nc.dram_tensor(f"zf{l}_{j}", [NCORES * PROW[j], DOUT[l]],
                                  ZDTL[l], kind="Internal",
                                  addr_space="Shared").ap()
                   for j in range(NPIECE))
          for l in (1, 2, 3, 4)}

    rg = [list(range(NCORES))]

    # column offsets of each group in gidx/DC/WW, and within-group tile
    # offsets: [selfs][seg0: blocks][seg1: blocks]
    gcol_of = np.concatenate([[0], np.cumsum(GT)]).astype(int)

    def seg_off(g, j):
        """Offset of seg j's tile run within group g (all blocks)."""
        B = GROUPS[g]
        return len(B) + int(T_seg[list(B), :j].sum())

    def blk_off(g, j, k):
        """Offset of block B[k]'s seg-j tile run within group g."""
        B = GROUPS[g]
        return seg_off(g, j) + int(T_seg[list(B[:k]), j].sum())

    with tile.TileContext(nc) as tc:
        with (
            tc.tile_pool(name="meta", bufs=8) as meta,
            tc.tile_pool(name="gp", bufs=7) as gp,
            tc.tile_pool(name="sp", bufs=6) as sp,
            tc.tile_pool(name="mk", bufs=2) as mk,
            tc.tile_pool(name="zp", bufs=3) as zp,
            tc.tile_pool(name="hp", bufs=3) as hp,
            tc.tile_pool(name="mp", bufs=3) as mp,
            tc.tile_pool(name="ps", bufs=2, space="PSUM") as ps,
            tc.tile_pool(name="psa", bufs=3, space="PSUM") as psa,
            tc.tile_pool(name="pst", bufs=2, space="PSUM") as pst,
        ):
            # ---- constant / weight loads (hT in column chunks so dense L1
            # can start early)
            # startup loads split across both HWDGE rings (scalar is otherwise
            # idle until the first agg phase) so dense L1 and the first
            # AllGather fire sooner
            nc.scalar.dma_start(W_sb[1], Wd[1].ap())
            nc.sync.dma_start(ident_sb, ident_c.ap())
            nc.scalar.dma_start(DC_sb, DCd.ap())
            nc.scalar.dma_start(WW_sb, WWd.ap())
            nc.scalar.dma_start(iota_sb, iota_c.ap())
            CCH = SHARD // 4
            for c in range(4):
                for k in range(KC):
                    eng = nc.sync if k % 2 == 0 else nc.scalar
                    eng.dma_start(
                        hT[k][:, c * CCH:(c + 1) * CCH],
                        xT[k * 128:(k + 1) * 128, c * CCH:(c + 1) * CCH])
            nc.sync.dma_start(W_sb[0], Wd[2].ap())
            nc.sync.dma_start(W4_sb, Wd[4].ap())
            for l in (1, 2, 3, 4):
                nc.sync.dma_start(Bb_sb[l], Bd[l].ap())
                nc.sync.dma_start(Brow_sb[l], Brd[l].ap())
            nc.vector.memset(ones_sb, 1.0)
            nc.sync.dma_start(M1_sb, M1p.ap())
            nc.sync.dma_start(M2_sb, M2d.ap())
            nc.sync.dma_start(M3_sb, M3d.ap())
            nc.sync.dma_start(MB1_sb, MB1.ap())
            nc.sync.dma_start(MB2_sb, MB2.ap())
            nc.sync.dma_start(MB3_sb, MB3b.ap())

            # SWDGE queue load balancing (greedy by tile count)
            qload = [0, 0, 0, 0]

            # zero the gather-buffer slots once: trimmed tail rows are never
            # gathered, and uninitialized SBUF could hold fp8 NaN patterns
            for _ in range(7):
                zt = gp.tile([128, GTMAX, DH], fp8, tag="gbuf", name="gbuf8")
                nc.vector.memset(zt[:], 0)

            def gbuf_tile(l):
                # All layers share one pool slot family (same bytes per slot):
                #   l=1..3: fp8  [128, GTMAX, 512] -> group tiles at 512B
                #   l=4:    bf16 [128, 2*GTMAX, 128] -> group tiles at 256B
                if l <= 3:
                    return gp.tile([128, GTMAX, DH], fp8, tag="gbuf",
                                   name="gbuf8")
                return gp.tile([128, 2 * GTMAX, 128], bf16, tag="gbuf",
                               name="gbuf4")

            def dense_tile(l, t):
                """z_l[t] = hT[:, t] @ W_l -> zdt -> DRAM z_loc."""
                Dout = DOUT[l]
                wsb = W_sb[l % 2] if l < 4 else W4_sb
                zps = ps.tile([128, DH], f32, tag="zps")
                for k in range(KC):
                    nc.tensor.matmul(
                        zps[:, 0:Dout],
                        hT[k][:, t * 128:(t + 1) * 128],
                        wsb[:, k * Dout:(k + 1) * Dout],
                        start=(k == 0), stop=(k == KC - 1),
                    )
                zsb = zp.tile([128, Dout], ZDTL[l], tag=f"zsb_{l}")
                nc.scalar.activation(zsb[:], zps[:, 0:Dout],
                                     mybir.ActivationFunctionType.Copy)
                nc.sync.dma_start(z_loc[l][t * 128:(t + 1) * 128, :], zsb[:])

            def fire_ag(l, k):
                """Fire sub-collective k (writes a row slice of its piece)."""
                j = SUBPIECE[k]
                nc.gpsimd.collective_compute(
                    "AllGather", mybir.AluOpType.bypass,
                    replica_groups=rg,
                    ins=[z_loc[l][SUBSTART[k]:SUBSTART[k] + SUBROW[k], :]],
                    outs=[zf[l][j][SUBBASE[k]:SUBBASE[k]
                                   + NCORES * SUBROW[k], :]],
                )

            def agg_load(l, g):
                """gbuf alloc + idx/S/self loads + seg-0 gathers for group g."""
                Dout = DOUT[l]
                B = GROUPS[g]
                TG = GT[g]
                col = int(gcol_of[g])
                gbuf = gbuf_tile(l)
                idx_sb = meta.tile([128, TG * 8], i16, tag="idx")
                nc.sync.dma_start(idx_sb[:], gidx.ap()[:, col * 8:(col + TG) * 8])
                # Build S on-chip (saves ~19MB of HBM one-hot traffic per
                # layer): mask = (iota == DC), S = mask * WW, batched over
                # the whole group's TG tiles with stride-0 broadcast APs.
                # (Caching built S in per-block DRAM tensors for layers 2-3
                # was tried twice and regressed both times (+103us, +35us):
                # DVE runs in parallel slack here -- GatherGen+DMA bind, and
                # the cache traffic re-pressures DMA.)
                sdt = fp8 if l <= 3 else bf16
                S_sb = sp.tile([128, TG, 128], sdt, tag="S8" if l <= 3 else "S")
                msk = mk.tile([128, TG, 128], f16, tag="msk")
                i3 = iota_sb.rearrange("p (a d) -> p a d", a=1)      # [128,1,128]
                d3 = DC_sb[:, col:col + TG]\
                    .rearrange("p (t a) -> p t a", a=1)              # [128,TG,1]
                ib, db = bass.broadcast_tensor_aps(i3, d3)
                nc.vector.tensor_tensor(msk[:], ib, db,
                                        mybir.AluOpType.is_equal)
                w3 = WW_sb[:, col:col + TG]\
                    .rearrange("p (t a) -> p t a", a=1)              # [128,TG,1]
                mb_, wb = bass.broadcast_tensor_aps(msk[:], w3)
                nc.vector.tensor_tensor(S_sb[:], mb_, wb,
                                        mybir.AluOpType.mult)

                # self tiles: one contiguous copy of the group's own z rows
                nb = len(B)
                nc.sync.dma_start(gbuf[:, 0:nb, :],
                                  z_loc[l][B[0] * 128:(B[0] + nb) * 128, :]
                                  .rearrange("(a p) d -> p a d", a=nb))
                ctx = (l, g, Dout, gbuf, idx_sb, S_sb)
                agg_gather(ctx, 0)
                return ctx

            def agg_gather(ctx, j):
                """Emit the seg-j gathers for group g (all blocks)."""
                l, g, Dout, gbuf, idx_sb, S_sb = ctx
                off = seg_off(g, j)
                # SWDGE descriptor ring holds ~1024 descs/queue (ucode
                # constant — it does NOT grow with dynamic_dma_scratch_size;
                # bigger calls hang the device).  Cap at 7 tiles (896 idxs).
                # num_idxs is trimmed to the worst-core true count (the tail
                # pads are never gathered; their S rows are zero and gbuf
                # slots are memset once at startup, so stale rows are finite).
                left = int(T_seg[list(GROUPS[g]), j].sum())
                rows = int(sum(cmax[b, j] for b in GROUPS[g]))
                while left > 0:
                    sub = min(7, left)
                    n = min(sub * 128, rows)
                    nt = (n + 127) // 128
                    qn = qload.index(min(qload))
                    qload[qn] += sub
                    nc.gpsimd.dma_gather(
                        gbuf[:, off:off + nt, :],
                        zf[l][j],
                        idx_sb[:, off * 8:(off + nt) * 8],
                        num_idxs=n, num_idxs_reg=n, elem_size=Dout,
                        queue_num=qn,
                    )
                    off += sub
                    left -= sub
                    rows -= n

            def agg_compute(ctx, k):
                """S.T @ gbuf accumulation + epilogue for block B[k] of g."""
                l, g, Dout, gbuf, idx_sb, S_sb = ctx
                b = GROUPS[g][k]
                # tile runs of this block: [self] + per-seg slices; merge
                # adjacent contiguous runs so DoubleRow pairs span them
                runs = [(k, 1)] + [
                    (blk_off(g, j, k), int(T_seg[b, j]))
                    for j in range(NPIECE)
                ]
                merged = [runs[0]]
                for off, cnt in runs[1:]:
                    po, pc = merged[-1]
                    if off == po + pc:
                        merged[-1] = (po, pc + cnt)
                    else:
                        merged.append((off, cnt))
                runs = merged
                # (Narrowing layer 4's matmuls to the 16 real feature cols
                # was tried and measured +130us: a 16-cycle stream can't hide
                # the 128-col LDWEIGHTS, so every matmul goes LDW-bound.)
                Dm = Dout
                aps = psa.tile([128, DH], f32, tag="aps")
                first = True
                for off, cnt in runs:
                    e = off
                    while e < off + cnt:
                        if l <= 3 and e + 1 < off + cnt:
                            # fp8: DoubleRow packs two 128-edge tiles per
                            # instruction (contracts 256 edges)
                            nc.tensor.matmul(
                                aps[:, 0:Dm],
                                S_sb[:, e:e + 2, :], gbuf[:, e:e + 2, :],
                                start=first, stop=False,
                                perf_mode=mybir.MatmulPerfMode.DoubleRow,
                            )
                            e += 2
                        else:
                            nc.tensor.matmul(
                                aps[:, 0:Dm], S_sb[:, e, :],
                                gbuf[:, e, 0:Dm],
                                start=first, stop=False,
                            )
                            e += 1
                        first = False
                # bias folded into the accumulation as a rank-1 matmul
                # (ones.T @ bias_row) so the relu can read PSUM directly
                nc.tensor.matmul(
                    aps[:, 0:Dm], ones_sb[0:1, 0:128],
                    Brow_sb[l][0:1, 0:Dm],
                    start=False, stop=True,
                )

                # epilogue: h = relu(agg + b) straight from PSUM (bias is
                # already accumulated); transpose back to feature-major
                hsb = hp.tile([128, Dm], f32, tag="hsb")
                nc.scalar.activation(hsb[:], aps[:, 0:Dm],
                                     mybir.ActivationFunctionType.Relu)
                for k in range(Dout // 128):
                    tps = pst.tile([128, 128], f32, tag="tps")
                    nc.tensor.transpose(tps[:],
                                        hsb[:, k * 128:(k + 1) * 128],
                                        ident_sb)
                    nc.scalar.activation(hT[k][:, b * 128:(b + 1) * 128],
                                         tps[:],
                                         mybir.ActivationFunctionType.Copy)

            def mlp_block(b):
                """out rows of block b from h5 = hT[0][:, b] (128-wide)."""
                sl = slice(b * 128, (b + 1) * 128)
                p5t = ps.tile([128, DH], f32, tag="zps")
                p5 = p5t[0:MH, 0:128]
                nc.tensor.matmul(p5, M1_sb, hT[0][:, sl],
                                 start=True, stop=True)
                h5 = mp.tile([MH, 128], f16, tag="h5")
                nc.scalar.activation(h5[:], p5,
                                     mybir.ActivationFunctionType.Relu,
                                     bias=MB1_sb)
                p6t = psa.tile([128, DH], f32, tag="aps")
                p6 = p6t[0:MH, 0:128]
                nc.tensor.matmul(p6, M2_sb, h5[:], start=True, stop=True)
                h6 = mp.tile([MH, 128], f16, tag="h6")
                nc.scalar.activation(h6[:], p6,
                                     mybir.ActivationFunctionType.Relu,
                                     bias=MB2_sb)
                pot = pst.tile([128, 128], f32, tag="tps")
                po = pot[:, 0:NCLS]
                nc.tensor.matmul(po, h6[:], M3_sb, start=True, stop=True)
                osb = zp.tile([128, NCLS], f32, tag="osb")
                nc.vector.tensor_tensor(osb[:], po, MB3_sb,
                                        mybir.AluOpType.add)
                nc.sync.dma_start(out.ap()[sl, :], osb[:])

            # ---- software pipeline over layers ----
            # sub-collective k fires once the dense tiles covering its z rows
            # are emitted (the last sub fires after the loop)
            FIRE_AT = {sum(SUBBLK[:k + 1]) - 1: k for k in range(NSUB - 1)}
            # layer 1 dense alone (reads x), AG pieces fired asap
            for t in range(NB):
                dense_tile(1, t)
                if t in FIRE_AT:
                    fire_ag(1, FIRE_AT[t])
            fire_ag(1, NSUB - 1)

            # At each phase start the previous layer's last AllGather piece is
            # still in flight; emit the seg-0 gathers (ready data) of the
            # first K groups before any seg-1 gather so the gpsimd queue and
            # HBM stay busy through the collective window.
            K = 7
            for l in (2, 3, 4):
                # preload layer-3 dense weights into the now-idle parity
                # buffer (W1/W2/W4 were loaded at startup)
                if l == 3:
                    nc.sync.dma_start(W_sb[1], Wd[3].ap())
                ctxs = [agg_load(l - 1, g) for g in range(K)]
                for g in range(NG):
                    ctx = ctxs[g] if g < K else agg_load(l - 1, g)
                    for j in range(1, NPIECE):
                        agg_gather(ctx, j)
                    for k, b in enumerate(GROUPS[g]):
                        agg_compute(ctx, k)
                        dense_tile(l, b)
                        if b in FIRE_AT:
                            fire_ag(l, FIRE_AT[b])
                fire_ag(l, NSUB - 1)

            # final aggregation of layer 4 feeding the MLP head per block
            ctxs = [agg_load(4, g) for g in range(K)]
            for g in range(NG):
                ctx = ctxs[g] if g < K else agg_load(4, g)
                for j in range(1, NPIECE):
                    agg_gather(ctx, j)
                for k, b in enumerate(GROUPS[g]):
                    agg_compute(ctx, k)
                    mlp_block(b)

    nc.compile()
    return nc


# ---------------------------------------------------------------- entry point

def kernel(x, edge_index, W1, b1, W2, b2, W3, b3, W4, b4,
           M1, mb1, M2, mb2, M3, mb3):
    global LAST_RESULT
    x = np.asarray(x, np.float32)
    edge_index = np.asarray(edge_index)
    meta = _host_prep(x, edge_index)
    key = (tuple(meta["T_seg"].reshape(-1).tolist()), meta["TT"],
           tuple(meta["cmax"].reshape(-1).tolist()))
    if key not in _BUILD_CACHE:
        _BUILD_CACHE[key] = _build(key[0], key[1], key[2])
    nc = _BUILD_CACHE[key]

    W4p = np.zeros((DIN, 128), np.float32)
    W4p[:, :DE] = np.asarray(W4, np.float32)
    b4p = np.zeros(128, np.float32)
    b4p[:DE] = np.asarray(b4, np.float32)
    M1p = np.zeros((128, MH), np.float32)
    M1p[:DE] = np.asarray(M1, np.float32)

    # fold the fp8 z scales into weights/biases: W_l' = (c_l/c_{l-1}) W_l,
    # b_l' = c_l b_l  (c_0 = 1; c_4 = 1 so h4 and the MLP are unscaled)
    c = ZSCALE
    Wch = {1: _chunk_w(np.asarray(W1, np.float32) * c[1]),
           2: _chunk_w(np.asarray(W2, np.float32) * (c[2] / c[1])),
           3: _chunk_w(np.asarray(W3, np.float32) * (c[3] / c[2])),
           4: _chunk_w(W4p * (c[4] / c[3]))}
    Bb = {1: np.broadcast_to(np.asarray(b1, np.float32) * c[1], (128, DH)).copy(),
          2: np.broadcast_to(np.asarray(b2, np.float32) * c[2], (128, DH)).copy(),
          3: np.broadcast_to(np.asarray(b3, np.float32) * c[3], (128, DH)).copy(),
          4: np.broadcast_to(b4p * c[4], (128, 128)).copy()}

    common = {
        **{f"W{l}": Wch[l] for l in (1, 2, 3, 4)},
        **{f"B{l}": Bb[l] for l in (1, 2, 3, 4)},
        **{f"Br{l}": np.ascontiguousarray(Bb[l][0:1]).astype(np.float16)
           for l in (1, 2, 3, 4)},
        "M1p": M1p.astype(np.float16),
        "M2d": np.asarray(M2, np.float16),
        "M3d": np.asarray(M3, np.float16),
        "MB1": np.asarray(mb1, np.float32).reshape(MH, 1),
        "MB2": np.asarray(mb2, np.float32).reshape(MH, 1),
        "MB3b": np.broadcast_to(np.asarray(mb3, np.float32), (128, NCLS)).copy(),
        "ident_c": np.eye(128, dtype=np.float32),
        "iota_c": np.broadcast_to(np.arange(128, dtype=np.float16),
                                  (128, 128)).copy(),
    }

    in_maps = []
    for r in range(NCORES):
        rows = min(SHARD, max(0, N - r * SHARD))
        xp = np.zeros((SHARD, DIN), np.float32)
        xp[:rows] = x[r * SHARD:r * SHARD + rows]
        in_maps.append({
            **common,
            "xT": np.ascontiguousarray(xp.T).astype(np.float16),
            "gidx": meta["gidx"][r],
            "DCd": meta["DC"][r],
            "WWd": meta["WW"][r],
        })

    LAST_RESULT = bass_utils.run_bass_kernel_spmd(
        nc, in_maps, core_ids=list(range(NCORES)),
    )
    out = np.concatenate([LAST_RESULT.results[r]["out"] for r in range(NCORES)], 0)
    return np.ascontiguousarray(out[:N]).astype(np.float32)

